# revision 10
# baseline (speedup 1.0000x reference)
"""CWICLinear (striped threshold-masked linear) TRN2 kernel.

Sharding: tensor-parallel over the 64 stripes -> 8 stripes per NeuronCore.
Each core holds out_features/8 = 1024 columns of weight plus the matching
threshold rows; x/mu are replicated; outputs are gathered on host.

Per-core engine split:
  ACT : xc = x - mu (in-place, per k block), psum evacuation with
        (post_mu + bias) per-partition bias
  DVE : a = |xc| (int32 mask), masked operand xm = (a > thr) * xc (fused stt)
  PE  : y matmuls (fp32, exact), rank-sum count matmuls (bf16)
  GPSIMD: mask count rank chain r += (a > thr_j) over the 8 local stripes
  SP  : DMAs (pieced ~0.25MB so single transfers don't serialize the start)
"""
import sys

sys.path.insert(0, "/opt/trn_rl_repo")

import numpy as np

IN_FEATURES = 2048
OUT_FEATURES = 8192
STRIPE_SIZE = 128
NUM_STRIPES = 64
N_CORES = 8
SPC = NUM_STRIPES // N_CORES      # stripes per core = 8
T = 512                           # tokens
P = 128                           # partitions
KB = IN_FEATURES // P             # k blocks = 16
FLOPS_CONST = float(IN_FEATURES) * float(OUT_FEATURES)  # 16777216.0

_NC_CACHE = {}
N_DVE_STRIPES = 3
PE_WARMUP = 0



def _build_nc():
    import concourse.bacc as bacc
    import concourse.mybir as mybir
    import concourse.tile as tile

    dt = mybir.dt
    af = mybir.ActivationFunctionType
    nc = bacc.Bacc(None, target_bir_lowering=False)

    xt_d = nc.declare_dram_parameter("xt", [P, KB, T], dt.float32, isOutput=False)
    w_d = nc.declare_dram_parameter("w", [SPC, P, KB * P], dt.float32, isOutput=False)
    thr_d = nc.declare_dram_parameter("thr", [P, SPC, KB], dt.float32, isOutput=False)
    nthr_d = nc.declare_dram_parameter("nthr", [P, SPC, KB], dt.float32, isOutput=False)
    mu_d = nc.declare_dram_parameter("mu", [P, KB], dt.float32, isOutput=False)
    pmb_d = nc.declare_dram_parameter("pmb", [P, SPC], dt.float32, isOutput=False)
    y_d = nc.declare_dram_parameter("y", [SPC, P, T], dt.float32, isOutput=True)
    cnt_d = nc.declare_dram_parameter("cnt", [1, T], dt.float32, isOutput=True)

    with tile.TileContext(nc) as tc:
        with (
            tc.tile_pool(name="xp", bufs=1) as xp,
            tc.tile_pool(name="wp", bufs=1) as wp,
            tc.tile_pool(name="cp", bufs=1) as cp,
            tc.tile_pool(name="mp", bufs=8) as mp,
            tc.tile_pool(name="op", bufs=3) as op,
            tc.tile_pool(name="py", bufs=3, space="PSUM") as pyp,
            tc.tile_pool(name="pc", bufs=1, space="PSUM") as pcp,
        ):
            # ---- small consts ----
            thr = cp.tile([P, SPC, KB], dt.float32, tag="thr")
            nc.sync.dma_start(thr[:], thr_d[:])
            nthr = cp.tile([P, SPC, KB], dt.float32, tag="nthr")
            nc.sync.dma_start(nthr[:], nthr_d[:])
            mu = cp.tile([P, KB], dt.float32, tag="mu")
            nc.sync.dma_start(mu[:], mu_d[:])
            pmb = cp.tile([P, SPC], dt.float32, tag="pmb")
            nc.sync.dma_start(pmb[:], pmb_d[:])

            # ---- pieced input DMAs, interleaved by need time ----
            xt = xp.tile([P, KB, T], dt.float32, tag="xt")    # becomes xc in-place
            wts = []
            for j in range(SPC):
                wtile = wp.tile([P, KB, P], dt.float32, tag=f"w{j}")
                wts.append(wtile)

            # stripe 0's W in kblk pieces (lowest latency), xt per kblk,
            # interleaved; remaining stripes in 4-kblk quarters.
            for k in range(KB):
                nc.sync.dma_start(xt[:, k, :], xt_d[:, k, :])
                nc.sync.dma_start(wts[0][:, k, :], w_d[0][:, k * P : (k + 1) * P])
            for j in range(1, SPC):
                for q in range(4):
                    nc.sync.dma_start(
                        wts[j][:, q * 4 : (q + 1) * 4, :],
                        w_d[j][:, q * 4 * P : (q + 1) * 4 * P],
                    )

            ones_8 = cp.tile([P, 2, 16], dt.float8e4, tag="ones")
            nc.vector.memset(ones_8[:], 1.0)

            if PE_WARMUP:
                junk = pcp.tile([1, 8], dt.float32, tag="junk")
                for wu in range(PE_WARMUP):
                    nc.tensor.matmul(
                        junk[:1, 0:1], ones_8[:, 0, 0:1], ones_8[:, 0, 0:1],
                        start=(wu == 0), stop=(wu == PE_WARMUP - 1),
                        skip_group_check=True,
                    )

            # ---- xc = x - mu in place on ACT; a = |xc| on DVE ----
            xc = xt
            at = xp.tile([P, KB, T], dt.float32, tag="at")
            for k in range(KB):
                nc.scalar.activation(
                    xc[:, k, :], xt[:, k, :], af.Identity,
                    bias=mu[:, k : k + 1], scale=1.0,
                )
                nc.scalar.activation(at[:, k, :], xc[:, k, :], af.Abs)

            psum_cm = pcp.tile([1, T], dt.float32, tag="cm")
            psum_cs = pcp.tile([1, T], dt.float32, tag="cs")
            ncm = [0]
            ncs = [0]

            # ---- main loop: masking (DVE) + m01 (DVE/ACT) + matmuls (PE) ----
            for j in range(SPC):
                wt = wts[j]
                psum_y = pyp.tile([P, T], dt.float32, tag="y")
                for k in range(KB):
                    xm = mp.tile([P, T], dt.float32, tag="xm")
                    nc.vector.scalar_tensor_tensor(
                        xm[:], at[:, k, :], thr[:, j, k : k + 1], xc[:, k, :],
                        mybir.AluOpType.is_gt, mybir.AluOpType.mult,
                    )
                    if k % 2 == 0:
                        m01p = mp.tile([P, 2, T], dt.float8e4, tag="m01p")
                    if j < N_DVE_STRIPES:
                        nc.vector.tensor_scalar(
                            m01p[:, k % 2, :], at[:, k, :], thr[:, j, k : k + 1], None,
                            mybir.AluOpType.is_gt,
                        )
                    else:
                        # Sign(a - thr) in {-1, 0, +1}; folded as (s + n)/2 below
                        nc.scalar.activation(
                            m01p[:, k % 2, :], at[:, k, :], af.Sign,
                            bias=nthr[:, j, k : k + 1], scale=1.0,
                        )
                    if k % 2 == 1:
                        if j < N_DVE_STRIPES:
                            pc, flag, tot = psum_cm, ncm, N_DVE_STRIPES * KB // 2
                        else:
                            pc, flag, tot = psum_cs, ncs, (SPC - N_DVE_STRIPES) * KB // 2
                        nc.tensor.matmul(
                            pc[:1, :], ones_8[:, :, 0:1], m01p[:],
                            start=(flag[0] == 0), stop=(flag[0] == tot - 1),
                            perf_mode=mybir.MatmulPerfMode.DoubleRow,
                            skip_group_check=True,
                        )
                        flag[0] += 1
                    nc.tensor.matmul(
                        psum_y[:], wt[:, k, :], xm[:],
                        start=(k == 0), stop=(k == KB - 1),
                        skip_group_check=True,
                    )
                # evac on ACT: y = psum_y + (post_mu + bias)[j]
                ysb = op.tile([P, T], dt.float32, tag="ysb")
                nc.scalar.activation(
                    ysb[:], psum_y[:], af.Identity,
                    bias=pmb[:, j : j + 1], scale=1.0,
                )
                for q in range(4):
                    nc.sync.dma_start(
                        y_d[j][:, q * 128 : (q + 1) * 128],
                        ysb[:, q * 128 : (q + 1) * 128],
                    )

            csb = op.tile([1, T], dt.float32, tag="csb")
            nsign = float((SPC - N_DVE_STRIPES) * KB * P)
            nc.vector.tensor_scalar(
                csb[:1, :], psum_cs[:1, :], nsign, 0.5,
                mybir.AluOpType.add, mybir.AluOpType.mult,
            )
            nc.vector.tensor_tensor(
                csb[:1, :], csb[:1, :], psum_cm[:1, :], mybir.AluOpType.add,
            )
            nc.sync.dma_start(cnt_d[:], csb[:1, :])

    nc.finalize()
    return nc


def _get_nc():
    if "nc" not in _NC_CACHE:
        _NC_CACHE["nc"] = _build_nc()
    return _NC_CACHE["nc"]


def _prep_in_maps(x, weight, bias, thresholds, mu, std):
    x = np.asarray(x, dtype=np.float32)
    weight = np.asarray(weight, dtype=np.float32)
    bias = np.asarray(bias, dtype=np.float32)
    thresholds = np.asarray(thresholds, dtype=np.float32)
    mu = np.asarray(mu, dtype=np.float32)
    std = np.asarray(std, dtype=np.float32)

    x2 = x.reshape(-1, IN_FEATURES)
    xt = np.ascontiguousarray(x2.T.reshape(KB, P, T).transpose(1, 0, 2))
    thresh = (thresholds * std[None, :]).astype(np.float32)
    thr_g = thresh.reshape(NUM_STRIPES, KB, P)
    # ACT computes xc = x + bias with bias = -mu
    neg_mu_sb = np.ascontiguousarray((-mu).reshape(KB, P).T)
    post_mu = (mu.astype(np.float64) @ weight.astype(np.float64))
    pmb_g = (post_mu + bias.astype(np.float64)).astype(np.float32).reshape(NUM_STRIPES, P)
    w4 = weight.reshape(KB, P, NUM_STRIPES, P)

    in_maps = []
    for c in range(N_CORES):
        g0 = c * SPC
        w_core = np.ascontiguousarray(
            w4[:, :, g0 : g0 + SPC, :].transpose(2, 1, 0, 3)
        ).reshape(SPC, P, KB * P)
        thr_core = np.ascontiguousarray(thr_g[g0 : g0 + SPC].transpose(2, 0, 1))
        pmb_core = np.ascontiguousarray(pmb_g[g0 : g0 + SPC].T)
        in_maps.append({
            "xt": xt, "w": w_core, "thr": thr_core, "nthr": -thr_core,
            "mu": neg_mu_sb, "pmb": pmb_core,
        })
    return in_maps


def kernel(x, weight, bias, thresholds, mu, std):
    from concourse.bass_utils import run_bass_kernel_spmd

    og_shape = np.asarray(x).shape[:-1]
    in_maps = _prep_in_maps(x, weight, bias, thresholds, mu, std)

    nc = _get_nc()
    res = run_bass_kernel_spmd(nc, in_maps, list(range(N_CORES)))

    y_parts = [res.results[c]["y"] for c in range(N_CORES)]
    y_all = np.concatenate(y_parts, axis=0)                     # [64, 128, 512]
    y_full = np.ascontiguousarray(y_all.reshape(OUT_FEATURES, T).T)
    y_out = y_full.reshape(*og_shape, OUT_FEATURES)

    cnt_total = np.zeros(T, dtype=np.float64)
    for c in range(N_CORES):
        cnt_total += res.results[c]["cnt"][0].astype(np.float64)
    flops_sparse = (cnt_total * (FLOPS_CONST / (NUM_STRIPES * IN_FEATURES)))
    flops_sparse = flops_sparse.astype(np.float32).reshape(*og_shape)
    flops_dense = np.full(og_shape, np.float32(FLOPS_CONST), dtype=np.float32)

    return y_out, flops_dense, flops_sparse
